# revision 1
# baseline (speedup 1.0000x reference)
"""Trainium2 Bass kernel for a dense transformer decoder block.

Sharding: sequence-parallel over the 4096 (B*T) rows -> 8 cores, 512 rows
each (batch = core//4, row block = core%4). No collectives: each core
recomputes full-batch self K/V and cross K/V (uniform SPMD program); the
causal structure is handled with a host-side row permutation (own rows
last) + per-partition exp bias (0 / -30) + a small triangular mask on the
4 diagonal s-tiles.

Layout: all activations live transposed [C(part-tiles), tokens(free)] so
every projection is lhsT=W (stationary), rhs=activation — no on-device
transposes. Softmax runs on scores^T [s, q]: exp on ScalarE with fused
1/sqrt(d) scale + per-partition mask bias; denominator via an appended
ones-column on V (row 64 of the PV accumulator = sum of probs).
Matmul operands are bf16, accumulation/residual/LN stats in fp32.
"""

import sys
import numpy as np

sys.path.insert(0, "/opt/trn_rl_repo")

import ml_dtypes  # noqa: E402
import concourse.bass as bass  # noqa: E402
import concourse.bacc as bacc  # noqa: E402
import concourse.tile as tile  # noqa: E402
from concourse import mybir  # noqa: E402
from concourse.bass_utils import run_bass_kernel_spmd  # noqa: E402

BF = ml_dtypes.bfloat16
F32 = mybir.dt.float32
BF16 = mybir.dt.bfloat16
AF = mybir.ActivationFunctionType
ALU = mybir.AluOpType

B, T, SE, C, H, HS = 2, 2048, 2048, 1024, 16, 64
NCORE = 8
RB = 512          # rows per core
KT = C // 128     # 8 k-tiles over C
ST = T // 128     # 16 s-tiles
EPS = 1e-5
NEG = -30.0
P = 128


def _build_nc():
    nc = bacc.Bacc(None, target_bir_lowering=False)

    def din(name, shape, dt=BF16):
        return nc.dram_tensor(name, shape, dt, kind="ExternalInput").ap()

    d = {}
    d["xT"] = din("xT", [C, T], BF16)         # permuted x^T (own rows last)
    d["xeT"] = din("xeT", [C, SE], BF16)      # x_e^T
    for n in ["wsq", "wsk", "wsv", "wcq", "wck", "wcv", "wmp", "wcp"]:
        d[n] = din(n, [C, C])
    d["wf1"] = din("wf1", [C, 4 * C])
    d["wf2"] = din("wf2", [4 * C, C])
    for n in ["bsq", "bsk", "bsv", "bcq", "bck", "bcv", "bmp", "bcp", "bf2",
              "g1", "t1", "g2", "t2", "g3", "t3"]:
        d[n] = din(n, [C], F32)
    d["bf1"] = din("bf1", [4 * C], F32)
    d["sbias"] = din("sbias", [T], F32)
    d["smask"] = din("smask", [RB, RB], BF16)
    d["outT"] = nc.dram_tensor("outT", [C, RB], F32, kind="ExternalOutput").ap()

    with tile.TileContext(nc) as tc:
        _emit(tc, nc, d)
    nc.finalize()
    return nc


def _emit(tc, nc, d):
    from contextlib import ExitStack
    ctx = ExitStack()
    ctx.enter_context(nc.allow_low_precision(reason="bf16 matmul operands"))

    # ---------------- persistent pools ----------------
    consts = ctx.enter_context(tc.tile_pool(name="consts", bufs=1))
    respool = ctx.enter_context(tc.tile_pool(name="respool", bufs=2))
    xnpool = ctx.enter_context(tc.tile_pool(name="xnpool", bufs=1))
    epool = ctx.enter_context(tc.tile_pool(name="epool", bufs=2))
    lnsmall = ctx.enter_context(tc.tile_pool(name="lnsmall", bufs=2))

    ones = consts.tile([P, P], BF16)
    nc.vector.memset(ones, 1.0)
    sb_sbias = consts.tile([P, ST], F32)
    nc.sync.dma_start(out=sb_sbias, in_=d["sbias"].rearrange("(st p) -> p st", p=P))
    sb_smask = consts.tile([P, 4, RB], BF16)
    smr = d["smask"].rearrange("(i p) q -> p i q", p=P)
    for i in range(4):
        nc.sync.dma_start(out=sb_smask[:, i, :], in_=smr[:, i, :])

    def colvec(name1d, n=KT):
        t = consts.tile([P, n], F32, tag=f"cv_{name1d}")
        nc.sync.dma_start(out=t, in_=d[name1d].rearrange("(m p) -> p m", p=P))
        return t

    cv = {n: colvec(n) for n in ["bsq", "bsk", "bsv", "bcq", "bck", "bcv",
                                 "bmp", "bcp", "bf2", "g1", "t1", "g2", "t2",
                                 "g3", "t3"]}
    cv["bf1"] = colvec("bf1", 32)

    def ln_apply(pools, src, src_is_bf, xn_out, g, b):
        """LayerNorm over C for RB token columns. src [P, KT, RB];
        writes xn_out [P, KT, RB] bf16."""
        pstat, pbc, sbtmp = pools
        s1 = pstat.tile([1, RB], F32, tag="s1")
        s2 = pstat.tile([1, RB], F32, tag="s1")
        for k in range(KT):
            if src_is_bf:
                xbk = src[:, k, :]
            else:
                xbk = epool.tile([P, RB], BF16, tag="xbk")
                nc.vector.tensor_copy(out=xbk, in_=src[:, k, :])
            xsqk = epool.tile([P, RB], BF16, tag="xsqk")
            nc.vector.tensor_mul(xsqk, xbk, xbk)
            nc.tensor.matmul(s1, ones[:, 0:1], xbk,
                             start=(k == 0), stop=(k == KT - 1))
            nc.tensor.matmul(s2, ones[:, 0:1], xsqk,
                             start=(k == 0), stop=(k == KT - 1))
        mu_f = lnsmall.tile([1, RB], BF16, tag="mu_f")
        mu_f2 = lnsmall.tile([1, RB], F32, tag="lntmp")
        var_f = lnsmall.tile([1, RB], F32, tag="var_f")
        rstd_bf = lnsmall.tile([1, RB], BF16, tag="rstd_bf")
        nc.vector.tensor_scalar_mul(mu_f, s1, 1.0 / C)
        nc.vector.tensor_scalar_mul(var_f, s2, 1.0 / C)
        nc.vector.tensor_mul(mu_f2, mu_f, mu_f)
        nc.vector.scalar_tensor_tensor(out=var_f, in0=var_f, scalar=EPS,
                                       in1=mu_f2, op0=ALU.add,
                                       op1=ALU.subtract)
        nc.scalar.activation(out=var_f, in_=var_f, func=AF.Sqrt, bias=0.0)
        nc.vector.reciprocal(rstd_bf, var_f)
        mu_ps = pbc.tile([P, RB], F32, tag="bc")
        rs_ps = pbc.tile([P, RB], F32, tag="bc")
        nc.tensor.matmul(mu_ps, ones[0:1, :], mu_f, start=True, stop=True)
        nc.tensor.matmul(rs_ps, ones[0:1, :], rstd_bf, start=True, stop=True)
        mu_sb = lnsmall.tile([P, RB], BF16, tag="mu_sb")
        rs_sb = lnsmall.tile([P, RB], BF16, tag="rs_sb")
        nc.vector.tensor_copy(out=mu_sb, in_=mu_ps)
        nc.vector.tensor_copy(out=rs_sb, in_=rs_ps)
        tmp = lnsmall.tile([P, RB], F32, tag="lntmp")
        for k in range(KT):
            nc.vector.tensor_sub(tmp, src[:, k, :], mu_sb)
            nc.vector.tensor_mul(tmp, tmp, rs_sb)
            nc.vector.tensor_scalar(out=xn_out[:, k, :], in0=tmp,
                                    scalar1=g[:, k:k + 1], scalar2=b[:, k:k + 1],
                                    op0=ALU.mult, op1=ALU.add)

    def proj_pass(ppk, wt_sb, bias, rhs_src, out_cb, cs):
        """out_cb[:, m, cs] = W.T @ rhs + bias for all 8 m-tiles (bf16 out)."""
        for m in range(KT):
            pk = ppk.tile([P, RB], F32, tag="pk")
            for k in range(KT):
                nc.tensor.matmul(pk, wt_sb[:, k, m * P:(m + 1) * P],
                                 rhs_src[:, k, :],
                                 start=(k == 0), stop=(k == KT - 1))
            nc.vector.tensor_scalar_add(out_cb[:, m, cs], pk, bias[:, m:m + 1])

    def build_v(ppv, vbuf, st, lhs_src, ss, w_sb):
        pv = ppv.tile([P, C], F32, tag="pv")
        for k in range(KT):
            for n2 in range(2):
                nc.tensor.matmul(pv[:, n2 * 512:(n2 + 1) * 512],
                                 lhs_src[:, k, ss * P:(ss + 1) * P],
                                 w_sb[:, k, n2 * 512:(n2 + 1) * 512],
                                 start=(k == 0), stop=(k == KT - 1))
        nc.vector.tensor_copy(out=vbuf[:, st, :, 0:HS],
                              in_=pv.rearrange("p (h d) -> p h d", h=H))
        nc.vector.memset(vbuf[:, st, :, HS:HS + 1], 1.0)

    def load_w(pool, name, tag="w"):
        t = pool.tile([P, KT, C], BF16, tag=tag)
        src = d[name].rearrange("(k p) m -> p k m", p=P)
        for k in range(KT):
            nc.sync.dma_start(out=t[:, k, :], in_=src[:, k, :])
        return t

    def dma_in_3d(dst, src):
        for k in range(dst.shape[1]):
            nc.sync.dma_start(out=dst[:, k, :], in_=src[:, k, :])

    def attention(qb, vb, kb, bv_cv, masked, htb):
        with tc.tile_pool(name="psc", bufs=2, space="PSUM") as psc, \
             tc.tile_pool(name="po", bufs=4, space="PSUM") as po:
            for hp in range(KT):
                sc = psc.tile([P, 2, RB], F32, tag="sc")
                o0 = po.tile([HS + 1, RB], F32, tag="o")
                o1 = po.tile([HS + 1, RB], F32, tag="o")
                otiles = (o0, o1)
                for st in range(ST):
                    for i in (0, 1):
                        nc.tensor.matmul(
                            sc[:, i, :],
                            kb[64 * i:64 * (i + 1), hp, st * P:(st + 1) * P],
                            qb[64 * i:64 * (i + 1), hp, :],
                            start=True, stop=True)
                    eb = epool.tile([P, 2, RB], BF16, tag="e")
                    bias_ap = sb_sbias[:, st:st + 1] if masked else sb_sbias[:, 15:16]
                    nc.scalar.activation(out=eb, in_=sc, func=AF.Exp,
                                         bias=bias_ap, scale=0.125)
                    if masked and st >= 12:
                        for i in (0, 1):
                            nc.vector.tensor_mul(eb[:, i, :], eb[:, i, :],
                                                 sb_smask[:, st - 12, :])
                    for i in (0, 1):
                        nc.tensor.matmul(otiles[i], vb[:, st, 2 * hp + i, :],
                                         eb[:, i, :],
                                         start=(st == 0), stop=(st == ST - 1))
                for i in (0, 1):
                    rzt = epool.tile([P, RB], BF16, tag="rzt")
                    nc.vector.reciprocal(rzt[HS:HS + 1, :],
                                         otiles[i][HS:HS + 1, :])
                    bc = po.tile([64, RB], F32, tag="o")
                    nc.tensor.matmul(bc, ones[HS:HS + 1, 0:64],
                                     rzt[HS:HS + 1, :], start=True, stop=True)
                    bcs = epool.tile([64, RB], BF16, tag="bcs")
                    nc.vector.tensor_copy(out=bcs, in_=bc)
                    if i == 0:
                        nc.vector.tensor_mul(htb[0:64, hp, :],
                                             otiles[i][0:HS, :], bcs)
                        nc.vector.tensor_scalar_add(
                            htb[0:64, hp, :], htb[0:64, hp, :],
                            bv_cv[0:64, hp:hp + 1])
                    else:
                        htmp = epool.tile([64, RB], BF16, tag="htmp")
                        nc.vector.tensor_mul(htmp, otiles[i][0:HS, :], bcs)
                        nc.vector.tensor_scalar_add(htmp, htmp,
                                                    bv_cv[64:128, hp:hp + 1])
                        nc.sync.dma_start(out=htb[64:128, hp, :], in_=htmp)

    def proj_residual(wname, bias_cv, htb, res_src_fn, res_out):
        """res_out[:,m,:] = W.T @ h + b + res_src_fn(m)."""
        with tc.tile_pool(name="pw_pr", bufs=1) as pw, \
             tc.tile_pool(name="ppr", bufs=2, space="PSUM") as ppm:
            w_sb = load_w(pw, wname, tag="wpr")
            for m in range(KT):
                pp = ppm.tile([P, RB], F32, tag="pp")
                for k in range(KT):
                    nc.tensor.matmul(pp, w_sb[:, k, m * P:(m + 1) * P],
                                     htb[:, k, :],
                                     start=(k == 0), stop=(k == KT - 1))
                nc.vector.tensor_scalar_add(res_out[:, m, :], pp,
                                            bias_cv[:, m:m + 1])
                nc.vector.tensor_add(res_out[:, m, :], res_out[:, m, :],
                                     res_src_fn(m))

    xTr = d["xT"].rearrange("(k p) s -> p k s", p=P)
    res1 = None
    res2 = None
    with tc.tile_pool(name="kpool", bufs=1) as kpool, \
         tc.tile_pool(name="vpool", bufs=1) as vpool, \
         tc.tile_pool(name="qpool", bufs=1) as qpool, \
         tc.tile_pool(name="hpool", bufs=1) as hpool, \
         tc.tile_pool(name="xocp", bufs=2) as xocp:
        kbuf = kpool.tile([P, KT, T], BF16, tag="k")
        vbuf = vpool.tile([P, ST, H, HS + 1], BF16, tag="v")
        qbuf = qpool.tile([P, KT, RB], BF16, tag="q")
        htb = hpool.tile([P, KT, RB], BF16, tag="h")

        # ======== Phase A: ln1 + self K/Q/V, single pass over 4 chunks ======
        with tc.tile_pool(name="pa_stat", bufs=2, space="PSUM") as pstat, \
             tc.tile_pool(name="pa_bc", bufs=2, space="PSUM") as pbc, \
             tc.tile_pool(name="pa_k", bufs=2, space="PSUM") as ppk, \
             tc.tile_pool(name="pa_v", bufs=1, space="PSUM") as ppv, \
             tc.tile_pool(name="pa_sb", bufs=1) as sbtmp, \
             tc.tile_pool(name="pa_w", bufs=2) as watmp:
            wsk_sb = load_w(watmp, "wsk")
            wsv_sb = load_w(watmp, "wsv")
            xn = None
            for c4 in range(4):
                cs = slice(c4 * RB, (c4 + 1) * RB)
                xb = sbtmp.tile([P, KT, RB], BF16, tag="xb")
                dma_in_3d(xb, xTr[:, :, cs])
                xn = sbtmp.tile([P, KT, RB], BF16, tag="xn")
                ln_apply((pstat, pbc, sbtmp), xb, True, xn, cv["g1"], cv["t1"])
                proj_pass(ppk, wsk_sb, cv["bsk"], xn, kbuf, cs)
                for ss in range(4):
                    build_v(ppv, vbuf, 4 * c4 + ss, xn, ss, wsv_sb)
            # Q from chunk 3's xn (own rows), after wsk slot is free
            wsq_sb = load_w(watmp, "wsq")
            proj_pass(ppk, wsq_sb, cv["bsq"], xn, qbuf, slice(0, RB))

        # ==================== self-attention + m_proj =======================
        attention(qbuf, vbuf, kbuf, cv["bsv"], True, htb)
        res1 = respool.tile([P, KT, RB], F32, tag="res")

        def self_res(m):
            xoc = xocp.tile([P, RB], BF16, tag="xoc")
            nc.sync.dma_start(out=xoc, in_=xTr[:, m, 3 * RB:T])
            return xoc

        proj_residual("wmp", cv["bmp"], htb, self_res, res1)

        # ===================== Phase C: cross-attention =====================
        with tc.tile_pool(name="pc_stat", bufs=2, space="PSUM") as pstat, \
             tc.tile_pool(name="pc_bc", bufs=2, space="PSUM") as pbc, \
             tc.tile_pool(name="pc_k", bufs=2, space="PSUM") as ppk, \
             tc.tile_pool(name="pc_v", bufs=1, space="PSUM") as ppv, \
             tc.tile_pool(name="pc_sb", bufs=2) as sbtmp, \
             tc.tile_pool(name="pc_w", bufs=1) as watmp:
            xn2 = xnpool.tile([P, KT, RB], BF16, tag="xn2")
            ln_apply((pstat, pbc, sbtmp), res1, False, xn2, cv["g2"], cv["t2"])
            wcq_sb = load_w(watmp, "wcq")
            proj_pass(ppk, wcq_sb, cv["bcq"], xn2, qbuf, slice(0, RB))
            xer = d["xeT"].rearrange("(k p) s -> p k s", p=P)
            wck_sb = load_w(watmp, "wck")
            for c4 in range(4):
                cs = slice(c4 * RB, (c4 + 1) * RB)
                xec = sbtmp.tile([P, KT, RB], BF16, tag="xec")
                dma_in_3d(xec, xer[:, :, cs])
                proj_pass(ppk, wck_sb, cv["bck"], xec, kbuf, cs)
            wcv_sb = load_w(watmp, "wcv")
            for c4 in range(4):
                cs = slice(c4 * RB, (c4 + 1) * RB)
                xec = sbtmp.tile([P, KT, RB], BF16, tag="xec")
                dma_in_3d(xec, xer[:, :, cs])
                for ss in range(4):
                    build_v(ppv, vbuf, 4 * c4 + ss, xec, ss, wcv_sb)

        attention(qbuf, vbuf, kbuf, cv["bcv"], False, htb)
        res2 = respool.tile([P, KT, RB], F32, tag="res")
        proj_residual("wcp", cv["bcp"], htb, lambda m: res1[:, m, :], res2)

    # ================================ FFN =================================
    with tc.tile_pool(name="pf_stat", bufs=2, space="PSUM") as pstat, \
         tc.tile_pool(name="pf_bc", bufs=2, space="PSUM") as pbc, \
         tc.tile_pool(name="pf_h", bufs=2, space="PSUM") as pph, \
         tc.tile_pool(name="pf_sb", bufs=2) as sbtmp, \
         tc.tile_pool(name="pf_w", bufs=2) as watmp, \
         tc.tile_pool(name="pf_h1", bufs=1) as h1pool:
        xn3 = xnpool.tile([P, KT, RB], BF16, tag="xn2")
        ln_apply((pstat, pbc, sbtmp), res2, False, xn3, cv["g3"], cv["t3"])
        h1 = h1pool.tile([P, 32, RB], BF16, tag="h1")
        wf1r = d["wf1"].rearrange("(k p) m -> p k m", p=P)
        for mg in range(4):
            wg = watmp.tile([P, KT, C], BF16, tag="w")
            dma_in_3d(wg, wf1r[:, :, mg * C:(mg + 1) * C])
            for mm in range(KT):
                m = mg * KT + mm
                pp = pph.tile([P, RB], F32, tag="pp")
                for k in range(KT):
                    nc.tensor.matmul(pp, wg[:, k, mm * P:(mm + 1) * P],
                                     xn3[:, k, :],
                                     start=(k == 0), stop=(k == KT - 1))
                nc.vector.tensor_scalar(out=h1[:, m, :], in0=pp,
                                        scalar1=cv["bf1"][:, m:m + 1],
                                        scalar2=0.0,
                                        op0=ALU.add, op1=ALU.max)
        wf2r = d["wf2"].rearrange("(k p) m -> p k m", p=P)
        outr = d["outT"].rearrange("(k p) q -> p k q", p=P)
        oT = respool.tile([P, KT, RB], F32, tag="res")
        for m in range(KT):
            wg2 = watmp.tile([P, 32, P], BF16, tag="w")
            src2 = wf2r[:, :, m * P:(m + 1) * P]
            for k4 in range(4):
                nc.sync.dma_start(out=wg2[:, 8 * k4:8 * (k4 + 1), :],
                                  in_=src2[:, 8 * k4:8 * (k4 + 1), :])
            pp = pph.tile([P, RB], F32, tag="pp")
            for k in range(32):
                nc.tensor.matmul(pp, wg2[:, k, :], h1[:, k, :],
                                 start=(k == 0), stop=(k == 31))
            nc.vector.tensor_scalar_add(oT[:, m, :], pp, cv["bf2"][:, m:m + 1])
            nc.vector.tensor_add(oT[:, m, :], oT[:, m, :], res2[:, m, :])
            nc.sync.dma_start(out=outr[:, m, :], in_=oT[:, m, :])

    ctx.close()


_NC_CACHE = None


def _get_nc():
    global _NC_CACHE
    if _NC_CACHE is None:
        _NC_CACHE = _build_nc()
    return _NC_CACHE


def _heads_concat(w):
    return np.ascontiguousarray(np.transpose(np.asarray(w), (1, 0, 2))
                                .reshape(C, C))


def kernel(**inputs):
    inp = {k: np.asarray(v) for k, v in inputs.items()}
    nc = _get_nc()

    shared = {
        "wsq": _heads_concat(inp["mq_w"]).astype(BF),
        "wsk": _heads_concat(inp["mk_w"]).astype(BF),
        "wsv": _heads_concat(inp["mv_w"]).astype(BF),
        "wcq": _heads_concat(inp["cq_w"]).astype(BF),
        "wck": _heads_concat(inp["ck_w"]).astype(BF),
        "wcv": _heads_concat(inp["cv_w"]).astype(BF),
        "wmp": inp["m_proj_w"].astype(BF),
        "wcp": inp["c_proj_w"].astype(BF),
        "wf1": inp["f_w1"].astype(BF),
        "wf2": inp["f_w2"].astype(BF),
        "bsq": inp["mq_b"].reshape(C).astype(np.float32),
        "bsk": inp["mk_b"].reshape(C).astype(np.float32),
        "bsv": inp["mv_b"].reshape(C).astype(np.float32),
        "bcq": inp["cq_b"].reshape(C).astype(np.float32),
        "bck": inp["ck_b"].reshape(C).astype(np.float32),
        "bcv": inp["cv_b"].reshape(C).astype(np.float32),
        "bmp": inp["m_proj_b"].astype(np.float32),
        "bcp": inp["c_proj_b"].astype(np.float32),
        "bf1": inp["f_b1"].astype(np.float32),
        "bf2": inp["f_b2"].astype(np.float32),
        "g1": inp["ln1_g"].astype(np.float32),
        "t1": inp["ln1_b"].astype(np.float32),
        "g2": inp["ln2_g"].astype(np.float32),
        "t2": inp["ln2_b"].astype(np.float32),
        "g3": inp["ln3_g"].astype(np.float32),
        "t3": inp["ln3_b"].astype(np.float32),
        "smask": np.triu(np.ones((RB, RB), np.float32)).astype(BF),
    }

    x = inp["x"].astype(np.float32)
    xe = inp["x_e"].astype(np.float32)
    in_maps = []
    for core in range(NCORE):
        b, j = core // 4, core % 4
        q0 = j * RB
        perm = np.concatenate([np.arange(0, q0),
                               np.arange(q0 + RB, T),
                               np.arange(q0, q0 + RB)])
        sb = np.zeros(T, np.float32)
        sb[q0:T - RB] = NEG
        m = dict(shared)
        m["xT"] = np.ascontiguousarray(x[b][perm].T).astype(BF)
        m["xeT"] = np.ascontiguousarray(xe[b].T).astype(BF)
        m["sbias"] = sb
        in_maps.append(m)

    res = run_bass_kernel_spmd(nc, in_maps, core_ids=list(range(NCORE)))
    out = np.empty((B, T, C), np.float32)
    for core in range(NCORE):
        b, j = core // 4, core % 4
        out[b, j * RB:(j + 1) * RB, :] = res.results[core]["outT"].T
    return out



# revision 6
# speedup vs baseline: 1.6532x; 1.6532x over previous
"""Trainium2 Bass kernel for a dense transformer decoder block.

Sharding: sequence-parallel over the 4096 (B*T) rows -> 8 cores, 512 rows
each (batch = core//4, row block = core%4). No collectives: each core
recomputes full-batch self K/V and cross K/V (uniform SPMD program); the
causal structure is handled with a host-side row permutation (own rows
last) + per-partition exp bias (0 / -30) + a small triangular mask on the
4 diagonal s-tiles.

v2 changes vs v1:
- All six Q/K/V projections (self + cross) run in fp8e4 with DoubleRow
  perf mode (2 k-subtiles per matmul, 0.5 cycles/row): weights are
  host-scaled by 32 into fp8, LN outputs are written fp8, and the PSUM
  epilogue rescales by 1/32.  m_proj / c_proj / FFN stay bf16 (their
  quantization error would land directly on the residual stream).
- Attention is software-pipelined: PV(st-1) is emitted after
  scores(st)+exp(st) so the Act-engine exp latency hides behind PE work,
  with two head-pairs of PSUM accumulators in flight.
- LayerNorm mean/rstd broadcasts run on GPSIMD (partition_broadcast)
  instead of PE matmuls + PSUM round-trips; softmax reciprocal broadcast
  likewise.
- Residual stream is bf16 (final output still fp32).
"""

import sys
import numpy as np

sys.path.insert(0, "/opt/trn_rl_repo")

import ml_dtypes  # noqa: E402
import concourse.bass as bass  # noqa: E402
import concourse.bacc as bacc  # noqa: E402
import concourse.tile as tile  # noqa: E402
from concourse import mybir  # noqa: E402
from concourse.bass_utils import run_bass_kernel_spmd  # noqa: E402

BF = ml_dtypes.bfloat16
F8 = ml_dtypes.float8_e4m3
F32 = mybir.dt.float32
BF16 = mybir.dt.bfloat16
FP8 = mybir.dt.float8e4
AF = mybir.ActivationFunctionType
ALU = mybir.AluOpType
DR = mybir.MatmulPerfMode.DoubleRow

B, T, SE, C, H, HS = 2, 2048, 2048, 1024, 16, 64
NCORE = 8
RB = 512          # rows per core
KT = C // 128     # 8 k-tiles over C
ST = T // 128     # 16 s-tiles
EPS = 1e-5
NEG = -30.0
P = 128
WS = 32.0         # host-side fp8 weight scale
IWS = 1.0 / WS


def _build_nc():
    nc = bacc.Bacc(None, target_bir_lowering=False)

    def din(name, shape, dt=BF16):
        return nc.dram_tensor(name, shape, dt, kind="ExternalInput").ap()

    d = {}
    d["xT"] = din("xT", [C, T], BF16)         # permuted x^T (own rows last)
    d["xeT"] = din("xeT", [C, SE], FP8)       # x_e^T (fp8 direct)
    for n in ["wsq", "wsk", "wsv", "wcq", "wck", "wcv"]:
        d[n] = din(n, [C, C], FP8)            # pre-scaled by WS
    for n in ["wmp", "wcp"]:
        d[n] = din(n, [C, C], BF16)
    d["wf1"] = din("wf1", [C, 4 * C], BF16)
    d["wf2"] = din("wf2", [4 * C, C], BF16)
    for n in ["bsq", "bsk", "bsv", "bcq", "bck", "bcv", "bmp", "bcp", "bf2",
              "g1", "t1", "g2", "t2", "g3", "t3"]:
        d[n] = din(n, [C], F32)
    d["bf1"] = din("bf1", [4 * C], F32)
    d["sbias"] = din("sbias", [T], F32)
    d["smask"] = din("smask", [RB, RB], BF16)
    d["outT"] = nc.dram_tensor("outT", [C, RB], F32, kind="ExternalOutput").ap()

    with tile.TileContext(nc) as tc:
        _emit(tc, nc, d)
    nc.finalize()
    return nc


def _emit(tc, nc, d):
    from contextlib import ExitStack
    ctx = ExitStack()
    ctx.enter_context(nc.allow_low_precision(reason="bf16/fp8 matmul operands"))

    # ---------------- persistent pools ----------------
    consts = ctx.enter_context(tc.tile_pool(name="consts", bufs=1))
    respool = ctx.enter_context(tc.tile_pool(name="respool", bufs=2))
    xnpool = ctx.enter_context(tc.tile_pool(name="xnpool", bufs=1))
    epool = ctx.enter_context(tc.tile_pool(name="epool", bufs=2))
    lnsmall = ctx.enter_context(tc.tile_pool(name="lnsmall", bufs=2))

    ones = consts.tile([P, P], BF16)
    nc.vector.memset(ones, 1.0)
    sb_sbias = consts.tile([P, ST], F32)
    nc.sync.dma_start(out=sb_sbias, in_=d["sbias"].rearrange("(st p) -> p st", p=P))
    sb_smask = consts.tile([P, 4, RB], BF16)
    smr = d["smask"].rearrange("(i p) q -> p i q", p=P)
    for i in range(4):
        nc.sync.dma_start(out=sb_smask[:, i, :], in_=smr[:, i, :])

    def colvec(name1d, n=KT):
        t = consts.tile([P, n], F32, tag=f"cv_{name1d}")
        nc.sync.dma_start(out=t, in_=d[name1d].rearrange("(m p) -> p m", p=P))
        return t

    cv = {n: colvec(n) for n in ["bsq", "bsk", "bsv", "bcq", "bck", "bcv",
                                 "bmp", "bcp", "bf2", "g1", "t1", "g2", "t2",
                                 "g3", "t3"]}
    cv["bf1"] = colvec("bf1", 32)

    def ln_apply(pools, src, xn_out, g, b):
        """LayerNorm over C for RB token columns. src [P, KT, RB] bf16;
        writes xn_out [P, KT, RB] (fp8 or bf16)."""
        pstat, sbtmp = pools
        s1 = pstat.tile([1, RB], F32, tag="s1")
        s2 = pstat.tile([1, RB], F32, tag="s1")
        xsq = sbtmp.tile([P, KT, RB], BF16, tag="xsq", bufs=1)
        nc.vector.tensor_mul(xsq, src, src)
        for k in range(KT):
            nc.tensor.matmul(s1, ones[:, 0:1], src[:, k, :],
                             start=(k == 0), stop=(k == KT - 1))
            nc.tensor.matmul(s2, ones[:, 0:1], xsq[:, k, :],
                             start=(k == 0), stop=(k == KT - 1))
        mu_f = lnsmall.tile([1, RB], BF16, tag="mu_f")
        mu_f2 = lnsmall.tile([1, RB], F32, tag="lntmp")
        var_f = lnsmall.tile([1, RB], F32, tag="var_f")
        rstd_bf = lnsmall.tile([1, RB], BF16, tag="rstd_bf")
        nc.vector.tensor_scalar_mul(mu_f, s1, 1.0 / C)
        nc.vector.tensor_scalar_mul(var_f, s2, 1.0 / C)
        nc.vector.tensor_mul(mu_f2, mu_f, mu_f)
        nc.vector.scalar_tensor_tensor(out=var_f, in0=var_f, scalar=EPS,
                                       in1=mu_f2, op0=ALU.add,
                                       op1=ALU.subtract)
        nc.scalar.activation(out=var_f, in_=var_f, func=AF.Sqrt, bias=0.0)
        nc.vector.reciprocal(rstd_bf, var_f)
        mu_sb = lnsmall.tile([P, RB], BF16, tag="mu_sb")
        rs_sb = lnsmall.tile([P, RB], BF16, tag="rs_sb")
        nc.gpsimd.partition_broadcast(mu_sb, mu_f)
        nc.gpsimd.partition_broadcast(rs_sb, rstd_bf)
        tmp = lnsmall.tile([P, RB], BF16, tag="lntmp2")
        for k in range(KT):
            nc.vector.tensor_sub(tmp, src[:, k, :], mu_sb)
            nc.vector.tensor_mul(tmp, tmp, rs_sb)
            nc.vector.tensor_scalar(out=xn_out[:, k, :], in0=tmp,
                                    scalar1=g[:, k:k + 1], scalar2=b[:, k:k + 1],
                                    op0=ALU.mult, op1=ALU.add)

    def proj_f8(ppk, w8, bias, rhs8, out_cb, cs):
        """out_cb[:, m, cs] = (W/WS).T @ rhs + bias, fp8 DoubleRow."""
        for m in range(KT):
            pk = ppk.tile([P, RB], F32, tag="pk")
            for k2 in range(KT // 2):
                nc.tensor.matmul(pk, w8[:, 2 * k2:2 * k2 + 2, m * P:(m + 1) * P],
                                 rhs8[:, 2 * k2:2 * k2 + 2, :],
                                 start=(k2 == 0), stop=(k2 == KT // 2 - 1),
                                 perf_mode=DR)
            nc.vector.tensor_scalar(out=out_cb[:, m, cs], in0=pk,
                                    scalar1=IWS, scalar2=bias[:, m:m + 1],
                                    op0=ALU.mult, op1=ALU.add)

    def build_v_f8(ppv, vbuf, st, lhs8, ss, w8):
        """vbuf[:, st, :, 0:HS] = ((x/1).T W/WS) for 128 tokens; fp8 DoubleRow
        with swapped operands so tokens land on partitions."""
        for half in range(2):
            pv = ppv.tile([P, RB], F32, tag="pk")
            for k2 in range(KT // 2):
                nc.tensor.matmul(pv,
                                 lhs8[:, 2 * k2:2 * k2 + 2, ss * P:(ss + 1) * P],
                                 w8[:, 2 * k2:2 * k2 + 2,
                                    half * 512:(half + 1) * 512],
                                 start=(k2 == 0), stop=(k2 == KT // 2 - 1),
                                 perf_mode=DR)
            nc.vector.tensor_scalar_mul(
                vbuf[:, st, 8 * half:8 * (half + 1), 0:HS],
                pv.rearrange("p (h d) -> p h d", h=8), IWS)

    def load_w(pool, name, tag="w", dt=BF16):
        t = pool.tile([P, KT, C], dt, tag=tag)
        src = d[name].rearrange("(k p) m -> p k m", p=P)
        for k in range(KT):
            nc.sync.dma_start(out=t[:, k, :], in_=src[:, k, :])
        return t

    def dma_in_3d(dst, src):
        for k in range(dst.shape[1]):
            nc.sync.dma_start(out=dst[:, k, :], in_=src[:, k, :])

    def attention(qb, vb, kb, bv_cv, masked, htb):
        """Pipelined: per global step g=(hp,st): emit scores(g)+exp(g), then
        PV(g-1); epilogue(hp) right after PV(hp,15)."""
        with tc.tile_pool(name="psc", bufs=2, space="PSUM") as psc, \
             tc.tile_pool(name="po", bufs=4, space="PSUM") as po:
            prev = None  # (hp, st, eb, otiles)
            otiles = None

            def emit_pv(hp0, st0, eb0, ot0):
                for i in (0, 1):
                    nc.tensor.matmul(ot0[i], vb[:, st0, 2 * hp0 + i, :],
                                     eb0[:, i, :],
                                     start=(st0 == 0), stop=(st0 == ST - 1))

            def epilogue(hp0, ot0):
                for i in (0, 1):
                    rz = lnsmall.tile([1, RB], BF16, tag="rz")
                    nc.vector.reciprocal(rz, ot0[i][HS:HS + 1, :])
                    rzb = epool.tile([HS, RB], BF16, tag="rzb")
                    nc.gpsimd.partition_broadcast(rzb, rz)
                    hs = htb[64 * i:64 * (i + 1), hp0, :]
                    nc.vector.tensor_mul(hs, ot0[i][0:HS, :], rzb)
                    nc.vector.tensor_scalar_add(
                        hs, hs, bv_cv[64 * i:64 * (i + 1), hp0:hp0 + 1])

            for g in range(KT * ST):
                hp, st = divmod(g, ST)
                if st == 0:
                    otiles = (po.tile([HS + 1, RB], F32, tag="o", name="o0"),
                              po.tile([HS + 1, RB], F32, tag="o", name="o1"))
                sc = psc.tile([P, 2, RB], F32, tag="sc")
                for i in (0, 1):
                    nc.tensor.matmul(
                        sc[:, i, :],
                        kb[64 * i:64 * (i + 1), hp, st * P:(st + 1) * P],
                        qb[64 * i:64 * (i + 1), hp, :],
                        start=True, stop=True)
                eb = epool.tile([P, 2, RB], BF16, tag="e")
                bias_ap = sb_sbias[:, st:st + 1] if masked else sb_sbias[:, 15:16]
                nc.scalar.activation(out=eb, in_=sc, func=AF.Exp,
                                     bias=bias_ap, scale=0.125)
                if masked and st >= 12:
                    for i in (0, 1):
                        nc.vector.tensor_mul(eb[:, i, :], eb[:, i, :],
                                             sb_smask[:, st - 12, :])
                if prev is not None:
                    hp0, st0, eb0, ot0 = prev
                    emit_pv(hp0, st0, eb0, ot0)
                    if st0 == ST - 1:
                        epilogue(hp0, ot0)
                prev = (hp, st, eb, otiles)
            hp0, st0, eb0, ot0 = prev
            emit_pv(hp0, st0, eb0, ot0)
            epilogue(hp0, ot0)

    def proj_residual(wname, bias_cv, htb, res_src_fn, res_out):
        """res_out[:,m,:] = W.T @ h + b + res_src_fn(m). bf16."""
        with tc.tile_pool(name="pw_pr", bufs=1) as pw, \
             tc.tile_pool(name="ppr", bufs=2, space="PSUM") as ppm:
            w_sb = load_w(pw, wname, tag="wpr")
            for m in range(KT):
                pp = ppm.tile([P, RB], F32, tag="pp")
                for k in range(KT):
                    nc.tensor.matmul(pp, w_sb[:, k, m * P:(m + 1) * P],
                                     htb[:, k, :],
                                     start=(k == 0), stop=(k == KT - 1))
                nc.vector.tensor_scalar_add(res_out[:, m, :], pp,
                                            bias_cv[:, m:m + 1])
                nc.vector.tensor_add(res_out[:, m, :], res_out[:, m, :],
                                     res_src_fn(m))

    xTr = d["xT"].rearrange("(k p) s -> p k s", p=P)
    res1 = None
    res2 = None
    with tc.tile_pool(name="kpool", bufs=1) as kpool, \
         tc.tile_pool(name="vpool", bufs=1) as vpool, \
         tc.tile_pool(name="qpool", bufs=1) as qpool, \
         tc.tile_pool(name="hpool", bufs=1) as hpool, \
         tc.tile_pool(name="xocp", bufs=2) as xocp:
        kbuf = kpool.tile([P, KT, T], BF16, tag="k")
        vbuf = vpool.tile([P, ST, H, HS + 1], BF16, tag="v")
        qbuf = qpool.tile([P, KT, RB], BF16, tag="q")
        htb = hpool.tile([P, KT, RB], BF16, tag="h")
        nc.vector.memset(vbuf[:, :, :, HS:HS + 1], 1.0)

        # ======== Phase A: ln1 + self K/Q/V, single pass over 4 chunks ======
        with tc.tile_pool(name="pa_stat", bufs=2, space="PSUM") as pstat, \
             tc.tile_pool(name="pa_k", bufs=4, space="PSUM") as ppk, \
             tc.tile_pool(name="pa_sb", bufs=1) as sbtmp, \
             tc.tile_pool(name="pa_w", bufs=2) as watmp:
            wsk8 = load_w(watmp, "wsk", dt=FP8)
            wsv8 = load_w(watmp, "wsv", dt=FP8)
            xn = None
            for c4 in range(4):
                cs = slice(c4 * RB, (c4 + 1) * RB)
                xb = sbtmp.tile([P, KT, RB], BF16, tag="xb")
                dma_in_3d(xb, xTr[:, :, cs])
                xn = sbtmp.tile([P, KT, RB], FP8, tag="xn")
                ln_apply((pstat, sbtmp), xb, xn, cv["g1"], cv["t1"])
                proj_f8(ppk, wsk8, cv["bsk"], xn, kbuf, cs)
                for ss in range(4):
                    build_v_f8(ppk, vbuf, 4 * c4 + ss, xn, ss, wsv8)
            # Q from chunk 3's xn (own rows), after wsk slot is free
            wsq8 = load_w(watmp, "wsq", dt=FP8)
            proj_f8(ppk, wsq8, cv["bsq"], xn, qbuf, slice(0, RB))

        # ==================== self-attention + m_proj =======================
        attention(qbuf, vbuf, kbuf, cv["bsv"], True, htb)
        res1 = respool.tile([P, KT, RB], BF16, tag="res")

        def self_res(m):
            xoc = xocp.tile([P, RB], BF16, tag="xoc")
            nc.sync.dma_start(out=xoc, in_=xTr[:, m, 3 * RB:T])
            return xoc

        proj_residual("wmp", cv["bmp"], htb, self_res, res1)

        # ===================== Phase C: cross-attention =====================
        with tc.tile_pool(name="pc_stat", bufs=2, space="PSUM") as pstat, \
             tc.tile_pool(name="pc_k", bufs=4, space="PSUM") as ppk, \
             tc.tile_pool(name="pc_sb", bufs=2) as sbtmp, \
             tc.tile_pool(name="pc_w", bufs=2) as watmp:
            xn2 = xnpool.tile([P, KT, RB], FP8, tag="xn2")
            ln_apply((pstat, sbtmp), res1, xn2, cv["g2"], cv["t2"])
            wcq8 = load_w(watmp, "wcq", dt=FP8)
            proj_f8(ppk, wcq8, cv["bcq"], xn2, qbuf, slice(0, RB))
            xer = d["xeT"].rearrange("(k p) s -> p k s", p=P)
            wck8 = load_w(watmp, "wck", dt=FP8)
            wcv8 = load_w(watmp, "wcv", dt=FP8)
            for c4 in range(4):
                cs = slice(c4 * RB, (c4 + 1) * RB)
                xec = sbtmp.tile([P, KT, RB], FP8, tag="xec")
                dma_in_3d(xec, xer[:, :, cs])
                proj_f8(ppk, wck8, cv["bck"], xec, kbuf, cs)
                for ss in range(4):
                    build_v_f8(ppk, vbuf, 4 * c4 + ss, xec, ss, wcv8)

        attention(qbuf, vbuf, kbuf, cv["bcv"], False, htb)
        res2 = respool.tile([P, KT, RB], BF16, tag="res")
        proj_residual("wcp", cv["bcp"], htb, lambda m: res1[:, m, :], res2)

    # ================================ FFN =================================
    with tc.tile_pool(name="pf_stat", bufs=2, space="PSUM") as pstat, \
         tc.tile_pool(name="pf_h", bufs=2, space="PSUM") as pph, \
         tc.tile_pool(name="pf_sb", bufs=2) as sbtmp, \
         tc.tile_pool(name="pf_w", bufs=2) as watmp, \
         tc.tile_pool(name="pf_h1", bufs=1) as h1pool:
        xn3 = xnpool.tile([P, KT, RB], BF16, tag="xn2")
        ln_apply((pstat, sbtmp), res2, xn3, cv["g3"], cv["t3"])
        h1 = h1pool.tile([P, 32, RB], BF16, tag="h1")
        wf1r = d["wf1"].rearrange("(k p) m -> p k m", p=P)
        for mg in range(4):
            wg = watmp.tile([P, KT, C], BF16, tag="w")
            dma_in_3d(wg, wf1r[:, :, mg * C:(mg + 1) * C])
            for mm in range(KT):
                m = mg * KT + mm
                pp = pph.tile([P, RB], F32, tag="pp")
                for k in range(KT):
                    nc.tensor.matmul(pp, wg[:, k, mm * P:(mm + 1) * P],
                                     xn3[:, k, :],
                                     start=(k == 0), stop=(k == KT - 1))
                nc.vector.tensor_scalar(out=h1[:, m, :], in0=pp,
                                        scalar1=cv["bf1"][:, m:m + 1],
                                        scalar2=0.0,
                                        op0=ALU.add, op1=ALU.max)
        wf2r = d["wf2"].rearrange("(k p) m -> p k m", p=P)
        outr = d["outT"].rearrange("(k p) q -> p k q", p=P)
        for m in range(KT):
            wg2 = watmp.tile([P, 32, P], BF16, tag="w")
            src2 = wf2r[:, :, m * P:(m + 1) * P]
            for k4 in range(4):
                nc.sync.dma_start(out=wg2[:, 8 * k4:8 * (k4 + 1), :],
                                  in_=src2[:, 8 * k4:8 * (k4 + 1), :])
            pp = pph.tile([P, RB], F32, tag="pp")
            for k in range(32):
                nc.tensor.matmul(pp, wg2[:, k, :], h1[:, k, :],
                                 start=(k == 0), stop=(k == 31))
            om = sbtmp.tile([P, RB], F32, tag="om")
            nc.vector.tensor_scalar_add(om, pp, cv["bf2"][:, m:m + 1])
            nc.vector.tensor_add(om, om, res2[:, m, :])
            nc.sync.dma_start(out=outr[:, m, :], in_=om)

    ctx.close()


_NC_CACHE = None


def _get_nc():
    global _NC_CACHE
    if _NC_CACHE is None:
        _NC_CACHE = _build_nc()
    return _NC_CACHE


def _heads_concat(w):
    return np.ascontiguousarray(np.transpose(np.asarray(w), (1, 0, 2))
                                .reshape(C, C))


def kernel(**inputs):
    inp = {k: np.asarray(v) for k, v in inputs.items()}
    nc = _get_nc()

    def f8w(a):
        return (np.asarray(a) * WS).astype(F8)

    shared = {
        "wsq": f8w(_heads_concat(inp["mq_w"])),
        "wsk": f8w(_heads_concat(inp["mk_w"])),
        "wsv": f8w(_heads_concat(inp["mv_w"])),
        "wcq": f8w(_heads_concat(inp["cq_w"])),
        "wck": f8w(_heads_concat(inp["ck_w"])),
        "wcv": f8w(_heads_concat(inp["cv_w"])),
        "wmp": inp["m_proj_w"].astype(BF),
        "wcp": inp["c_proj_w"].astype(BF),
        "wf1": inp["f_w1"].astype(BF),
        "wf2": inp["f_w2"].astype(BF),
        "bsq": inp["mq_b"].reshape(C).astype(np.float32),
        "bsk": inp["mk_b"].reshape(C).astype(np.float32),
        "bsv": inp["mv_b"].reshape(C).astype(np.float32),
        "bcq": inp["cq_b"].reshape(C).astype(np.float32),
        "bck": inp["ck_b"].reshape(C).astype(np.float32),
        "bcv": inp["cv_b"].reshape(C).astype(np.float32),
        "bmp": inp["m_proj_b"].astype(np.float32),
        "bcp": inp["c_proj_b"].astype(np.float32),
        "bf1": inp["f_b1"].astype(np.float32),
        "bf2": inp["f_b2"].astype(np.float32),
        "g1": inp["ln1_g"].astype(np.float32),
        "t1": inp["ln1_b"].astype(np.float32),
        "g2": inp["ln2_g"].astype(np.float32),
        "t2": inp["ln2_b"].astype(np.float32),
        "g3": inp["ln3_g"].astype(np.float32),
        "t3": inp["ln3_b"].astype(np.float32),
        "smask": np.triu(np.ones((RB, RB), np.float32)).astype(BF),
    }

    x = inp["x"].astype(np.float32)
    xe = inp["x_e"].astype(np.float32)
    in_maps = []
    for core in range(NCORE):
        b, j = core // 4, core % 4
        q0 = j * RB
        perm = np.concatenate([np.arange(0, q0),
                               np.arange(q0 + RB, T),
                               np.arange(q0, q0 + RB)])
        sb = np.zeros(T, np.float32)
        sb[q0:T - RB] = NEG
        m = dict(shared)
        m["xT"] = np.ascontiguousarray(x[b][perm].T).astype(BF)
        m["xeT"] = np.ascontiguousarray(xe[b].T).astype(F8)
        m["sbias"] = sb
        in_maps.append(m)

    res = run_bass_kernel_spmd(nc, in_maps, core_ids=list(range(NCORE)))
    out = np.empty((B, T, C), np.float32)
    for core in range(NCORE):
        b, j = core // 4, core % 4
        out[b, j * RB:(j + 1) * RB, :] = res.results[core]["outT"].T
    return out


# revision 12
# speedup vs baseline: 1.8346x; 1.1097x over previous
"""Trainium2 Bass kernel for a dense transformer decoder block.

Sharding: sequence-parallel over the 4096 (B*T) rows -> 8 cores, 512 rows
each (batch = core//4, row block = core%4). No collectives: each core
recomputes full-batch self K/V and cross K/V (uniform SPMD program); the
causal structure is handled with a host-side row permutation (own rows
last) + per-partition exp bias (0 / -30) + a small triangular mask on the
4 diagonal s-tiles.

v2 changes vs v1:
- All six Q/K/V projections (self + cross) run in fp8e4 with DoubleRow
  perf mode (2 k-subtiles per matmul, 0.5 cycles/row): weights are
  host-scaled by 32 into fp8, LN outputs are written fp8, and the PSUM
  epilogue rescales by 1/32.  m_proj / c_proj / FFN stay bf16 (their
  quantization error would land directly on the residual stream).
- Attention is software-pipelined: PV(st-1) is emitted after
  scores(st)+exp(st) so the Act-engine exp latency hides behind PE work,
  with two head-pairs of PSUM accumulators in flight.
- LayerNorm mean/rstd broadcasts run on GPSIMD (partition_broadcast)
  instead of PE matmuls + PSUM round-trips; softmax reciprocal broadcast
  likewise.
- Residual stream is bf16 (final output still fp32).
"""

import sys
import numpy as np

sys.path.insert(0, "/opt/trn_rl_repo")

import ml_dtypes  # noqa: E402
import concourse.bass as bass  # noqa: E402
import concourse.bacc as bacc  # noqa: E402
import concourse.tile as tile  # noqa: E402
from concourse import mybir  # noqa: E402
from concourse.bass_utils import run_bass_kernel_spmd  # noqa: E402

BF = ml_dtypes.bfloat16
F8 = ml_dtypes.float8_e4m3
F32 = mybir.dt.float32
BF16 = mybir.dt.bfloat16
FP8 = mybir.dt.float8e4
AF = mybir.ActivationFunctionType
ALU = mybir.AluOpType
DR = mybir.MatmulPerfMode.DoubleRow

B, T, SE, C, H, HS = 2, 2048, 2048, 1024, 16, 64
NCORE = 8
RB = 512          # rows per core
KT = C // 128     # 8 k-tiles over C
ST = T // 128     # 16 s-tiles
EPS = 1e-5
NEG = -30.0
P = 128
WS = 32.0         # host-side fp8 weight scale
IWS = 1.0 / WS


def _build_nc():
    nc = bacc.Bacc(None, target_bir_lowering=False)

    def din(name, shape, dt=BF16):
        return nc.dram_tensor(name, shape, dt, kind="ExternalInput").ap()

    d = {}
    d["xT"] = din("xT", [C, T], BF16)         # permuted x^T (own rows last)
    d["xeT"] = din("xeT", [C, SE], FP8)       # x_e^T (fp8 direct)
    for n in ["wsq", "wsk", "wsv", "wcq", "wck", "wcv"]:
        d[n] = din(n, [C, C], FP8)            # pre-scaled by WS
    for n in ["wmp", "wcp"]:
        d[n] = din(n, [C, C], BF16)
    d["wf1"] = din("wf1", [C, 4 * C], BF16)
    d["wf2"] = din("wf2", [4 * C, C], BF16)
    for n in ["bsq", "bsk", "bsv", "bcq", "bck", "bcv", "bmp", "bcp", "bf2",
              "g1", "t1", "g2", "t2", "g3", "t3"]:
        d[n] = din(n, [C], F32)
    d["bf1"] = din("bf1", [4 * C], F32)
    d["sbias"] = din("sbias", [T], F32)
    d["smask"] = din("smask", [RB, RB], BF16)
    d["outT"] = nc.dram_tensor("outT", [C, RB], F32, kind="ExternalOutput").ap()

    with tile.TileContext(nc) as tc:
        _emit(tc, nc, d)
    nc.finalize()
    return nc


def _emit(tc, nc, d):
    from contextlib import ExitStack
    ctx = ExitStack()
    ctx.enter_context(nc.allow_low_precision(reason="bf16/fp8 matmul operands"))

    # ---------------- persistent pools ----------------
    consts = ctx.enter_context(tc.tile_pool(name="consts", bufs=1))
    respool = ctx.enter_context(tc.tile_pool(name="respool", bufs=2))
    xnpool = ctx.enter_context(tc.tile_pool(name="xnpool", bufs=1))
    epool = ctx.enter_context(tc.tile_pool(name="epool", bufs=2))
    lnsmall = ctx.enter_context(tc.tile_pool(name="lnsmall", bufs=2))

    ones = consts.tile([P, P], BF16)
    nc.vector.memset(ones, 1.0)
    sb_sbias = consts.tile([P, ST], F32)
    nc.sync.dma_start(out=sb_sbias, in_=d["sbias"].rearrange("(st p) -> p st", p=P))
    sb_smask = consts.tile([P, 4, RB], BF16)
    smr = d["smask"].rearrange("(i p) q -> p i q", p=P)
    for i in range(4):
        nc.sync.dma_start(out=sb_smask[:, i, :], in_=smr[:, i, :])

    def colvec(name1d, n=KT):
        t = consts.tile([P, n], F32, tag=f"cv_{name1d}")
        nc.sync.dma_start(out=t, in_=d[name1d].rearrange("(m p) -> p m", p=P))
        return t

    cv = {n: colvec(n) for n in ["bsq", "bsk", "bsv", "bcq", "bck", "bcv",
                                 "bmp", "bcp", "bf2", "g1", "t1", "g2", "t2",
                                 "g3", "t3"]}
    cv["bf1"] = colvec("bf1", 32)

    def ln_apply(pools, src, xn_out, g, b):
        """LayerNorm over C for RB token columns. src [P, KT, RB] bf16;
        writes xn_out [P, KT, RB] (fp8 or bf16)."""
        pstat, sbtmp = pools
        s1 = pstat.tile([1, RB], F32, tag="s1")
        s2 = pstat.tile([1, RB], F32, tag="s1")
        xsq = sbtmp.tile([P, KT, RB], BF16, tag="xsq", bufs=1)
        nc.vector.tensor_mul(xsq, src, src)
        for k in range(KT):
            nc.tensor.matmul(s1, ones[:, 0:1], src[:, k, :],
                             start=(k == 0), stop=(k == KT - 1))
            nc.tensor.matmul(s2, ones[:, 0:1], xsq[:, k, :],
                             start=(k == 0), stop=(k == KT - 1))
        mu_f = lnsmall.tile([1, RB], BF16, tag="mu_f")
        mu_f2 = lnsmall.tile([1, RB], F32, tag="lntmp")
        var_f = lnsmall.tile([1, RB], F32, tag="var_f")
        rstd_bf = lnsmall.tile([1, RB], BF16, tag="rstd_bf")
        nc.vector.tensor_scalar_mul(mu_f, s1, 1.0 / C)
        nc.vector.tensor_scalar_mul(var_f, s2, 1.0 / C)
        nc.vector.tensor_mul(mu_f2, mu_f, mu_f)
        nc.vector.scalar_tensor_tensor(out=var_f, in0=var_f, scalar=EPS,
                                       in1=mu_f2, op0=ALU.add,
                                       op1=ALU.subtract)
        nc.scalar.activation(out=var_f, in_=var_f, func=AF.Sqrt, bias=0.0)
        nc.vector.reciprocal(rstd_bf, var_f)
        mu_sb = lnsmall.tile([P, RB], BF16, tag="mu_sb")
        rs_sb = lnsmall.tile([P, RB], BF16, tag="rs_sb")
        nc.gpsimd.partition_broadcast(mu_sb, mu_f)
        nc.gpsimd.partition_broadcast(rs_sb, rstd_bf)
        for k in range(KT):
            tmp = lnsmall.tile([P, RB], BF16, tag="lntmp2")
            nc.vector.tensor_sub(tmp, src[:, k, :], mu_sb)
            nc.vector.tensor_mul(tmp, tmp, rs_sb)
            nc.scalar.activation(out=xn_out[:, k, :], in_=tmp,
                                 func=AF.Identity,
                                 bias=b[:, k:k + 1], scale=g[:, k:k + 1])

    def drain_sb(out, in_, scale, bias_ap, alt):
        """PSUM->SBUF drain with scale+bias, alternating Act/DVE by `alt`."""
        if alt % 2 == 0:
            nc.scalar.activation(out=out, in_=in_, func=AF.Identity,
                                 bias=bias_ap, scale=scale)
        else:
            nc.vector.tensor_scalar(out=out, in0=in_, scalar1=scale,
                                    scalar2=bias_ap, op0=ALU.mult, op1=ALU.add)

    def proj_f8(ppk, w8, bias, rhs8, out_cb, cs):
        """out_cb[:, m, cs] = (W/WS).T @ rhs + bias, fp8 DoubleRow."""
        for m in range(KT):
            pk = ppk.tile([P, RB], F32, tag="pk")
            for k2 in range(KT // 2):
                nc.tensor.matmul(pk, w8[:, 2 * k2:2 * k2 + 2, m * P:(m + 1) * P],
                                 rhs8[:, 2 * k2:2 * k2 + 2, :],
                                 start=(k2 == 0), stop=(k2 == KT // 2 - 1),
                                 perf_mode=DR)
            drain_sb(out_cb[:, m, cs], pk, IWS, bias[:, m:m + 1], m)

    def build_v_f8(ppv, vbuf, st, lhs8, ss, w8):
        """vbuf[:, st, :, 0:HS] = ((x/1).T W/WS) for 128 tokens; fp8 DoubleRow
        with swapped operands so tokens land on partitions."""
        for half in range(2):
            pv = ppv.tile([P, RB], F32, tag="pk")
            for k2 in range(KT // 2):
                nc.tensor.matmul(pv,
                                 lhs8[:, 2 * k2:2 * k2 + 2, ss * P:(ss + 1) * P],
                                 w8[:, 2 * k2:2 * k2 + 2,
                                    half * 512:(half + 1) * 512],
                                 start=(k2 == 0), stop=(k2 == KT // 2 - 1),
                                 perf_mode=DR)
            pvr = pv.rearrange("p (h d) -> p h d", h=8)
            vslice = vbuf[:, st, 8 * half:8 * (half + 1), 0:HS]
            if (st + half) % 2 == 0:
                nc.scalar.mul(vslice, pvr, IWS)
            else:
                nc.vector.tensor_scalar_mul(vslice, pvr, IWS)

    def load_w(pool, name, tag="w", dt=BF16):
        t = pool.tile([P, KT, C], dt, tag=tag)
        src = d[name].rearrange("(k p) m -> p k m", p=P)
        for k in range(KT):
            nc.sync.dma_start(out=t[:, k, :], in_=src[:, k, :])
        return t

    def dma_in_3d(dst, src):
        for k in range(dst.shape[1]):
            nc.sync.dma_start(out=dst[:, k, :], in_=src[:, k, :])

    def attention(qb, vb, kb, bv_cv, masked, htb):
        """Pipelined: per global step g=(hp,st): emit scores(g)+exp(g), then
        PV(g-1); epilogue(hp) right after PV(hp,15)."""
        with tc.tile_pool(name="psc", bufs=2, space="PSUM") as psc, \
             tc.tile_pool(name="po", bufs=4, space="PSUM") as po:
            prev = None  # (hp, st, eb, otiles)
            otiles = None

            def emit_pv(hp0, st0, eb0, ot0):
                for i in (0, 1):
                    nc.tensor.matmul(ot0[i], vb[:, st0, 2 * hp0 + i, :],
                                     eb0[:, i, :],
                                     start=(st0 == 0), stop=(st0 == ST - 1))

            def epilogue(hp0, ot0):
                for i in (0, 1):
                    rz = lnsmall.tile([1, RB], BF16, tag="rz")
                    nc.vector.reciprocal(rz, ot0[i][HS:HS + 1, :])
                    rzb = epool.tile([HS, RB], BF16, tag="rzb")
                    nc.gpsimd.partition_broadcast(rzb, rz)
                    hs = htb[64 * i:64 * (i + 1), hp0, :]
                    nc.vector.tensor_mul(hs, ot0[i][0:HS, :], rzb)
                    nc.vector.tensor_scalar_add(
                        hs, hs, bv_cv[64 * i:64 * (i + 1), hp0:hp0 + 1])

            for g in range(KT * ST):
                hp, st = divmod(g, ST)
                if st == 0:
                    otiles = (po.tile([HS + 1, RB], F32, tag="o", name="o0"),
                              po.tile([HS + 1, RB], F32, tag="o", name="o1"))
                sc = psc.tile([P, 2, RB], F32, tag="sc")
                for i in (0, 1):
                    nc.tensor.matmul(
                        sc[:, i, :],
                        kb[64 * i:64 * (i + 1), hp, st * P:(st + 1) * P],
                        qb[64 * i:64 * (i + 1), hp, :],
                        start=True, stop=True)
                eb = epool.tile([P, 2, RB], BF16, tag="e")
                bias_ap = sb_sbias[:, st:st + 1] if masked else sb_sbias[:, 15:16]
                nc.scalar.activation(out=eb, in_=sc, func=AF.Exp,
                                     bias=bias_ap, scale=0.125)
                if masked and st >= 12:
                    for i in (0, 1):
                        nc.vector.tensor_mul(eb[:, i, :], eb[:, i, :],
                                             sb_smask[:, st - 12, :])
                if prev is not None:
                    hp0, st0, eb0, ot0 = prev
                    emit_pv(hp0, st0, eb0, ot0)
                    if st0 == ST - 1:
                        epilogue(hp0, ot0)
                prev = (hp, st, eb, otiles)
            hp0, st0, eb0, ot0 = prev
            emit_pv(hp0, st0, eb0, ot0)
            epilogue(hp0, ot0)

    def proj_residual(wname, bias_cv, htb, res_src_fn, res_out):
        """res_out[:,m,:] = W.T @ h + b + res_src_fn(m). bf16."""
        with tc.tile_pool(name="pw_pr", bufs=1) as pw, \
             tc.tile_pool(name="ppr", bufs=2, space="PSUM") as ppm:
            w_sb = load_w(pw, wname, tag="wpr")
            for m in range(KT):
                pp = ppm.tile([P, RB], F32, tag="pp")
                for k in range(KT):
                    nc.tensor.matmul(pp, w_sb[:, k, m * P:(m + 1) * P],
                                     htb[:, k, :],
                                     start=(k == 0), stop=(k == KT - 1))
                drain_sb(res_out[:, m, :], pp, 1.0, bias_cv[:, m:m + 1], m)
                nc.vector.tensor_add(res_out[:, m, :], res_out[:, m, :],
                                     res_src_fn(m))

    xTr = d["xT"].rearrange("(k p) s -> p k s", p=P)
    res1 = None
    res2 = None
    with tc.tile_pool(name="kpool", bufs=1) as kpool, \
         tc.tile_pool(name="vpool", bufs=1) as vpool, \
         tc.tile_pool(name="qpool", bufs=1) as qpool, \
         tc.tile_pool(name="hpool", bufs=1) as hpool, \
         tc.tile_pool(name="xocp", bufs=2) as xocp:
        kbuf = kpool.tile([P, KT, T], BF16, tag="k")
        vbuf = vpool.tile([P, ST, H, HS + 1], BF16, tag="v")
        qbuf = qpool.tile([P, KT, RB], BF16, tag="q")
        htb = hpool.tile([P, KT, RB], BF16, tag="h")
        nc.vector.memset(vbuf[:, :, :, HS:HS + 1], 1.0)

        # ======== Phase A: ln1 + self K/Q/V, single pass over 4 chunks ======
        with tc.tile_pool(name="pa_stat", bufs=2, space="PSUM") as pstat, \
             tc.tile_pool(name="pa_k", bufs=4, space="PSUM") as ppk, \
             tc.tile_pool(name="pa_sb", bufs=1) as sbtmp, \
             tc.tile_pool(name="pa_w", bufs=2) as watmp:
            wsk8 = load_w(watmp, "wsk", dt=FP8)
            wsv8 = load_w(watmp, "wsv", dt=FP8)
            xn = None
            for c4 in range(4):
                cs = slice(c4 * RB, (c4 + 1) * RB)
                xb = sbtmp.tile([P, KT, RB], BF16, tag="xb", bufs=2)
                dma_in_3d(xb, xTr[:, :, cs])
                xn = sbtmp.tile([P, KT, RB], FP8, tag="xn", bufs=2)
                ln_apply((pstat, sbtmp), xb, xn, cv["g1"], cv["t1"])
                proj_f8(ppk, wsk8, cv["bsk"], xn, kbuf, cs)
                for ss in range(4):
                    build_v_f8(ppk, vbuf, 4 * c4 + ss, xn, ss, wsv8)
            # Q from chunk 3's xn (own rows), after wsk slot is free
            wsq8 = load_w(watmp, "wsq", dt=FP8)
            proj_f8(ppk, wsq8, cv["bsq"], xn, qbuf, slice(0, RB))

        # ==================== self-attention + m_proj =======================
        attention(qbuf, vbuf, kbuf, cv["bsv"], True, htb)
        res1 = respool.tile([P, KT, RB], BF16, tag="res")

        def self_res(m):
            xoc = xocp.tile([P, RB], BF16, tag="xoc")
            nc.sync.dma_start(out=xoc, in_=xTr[:, m, 3 * RB:T])
            return xoc

        proj_residual("wmp", cv["bmp"], htb, self_res, res1)

        # ===================== Phase C: cross-attention =====================
        with tc.tile_pool(name="pc_stat", bufs=2, space="PSUM") as pstat, \
             tc.tile_pool(name="pc_k", bufs=4, space="PSUM") as ppk, \
             tc.tile_pool(name="pc_sb", bufs=2) as sbtmp, \
             tc.tile_pool(name="pc_w", bufs=2) as watmp:
            xn2 = xnpool.tile([P, KT, RB], FP8, tag="xn2")
            ln_apply((pstat, sbtmp), res1, xn2, cv["g2"], cv["t2"])
            wcq8 = load_w(watmp, "wcq", dt=FP8)
            proj_f8(ppk, wcq8, cv["bcq"], xn2, qbuf, slice(0, RB))
            xer = d["xeT"].rearrange("(k p) s -> p k s", p=P)
            wck8 = load_w(watmp, "wck", dt=FP8)
            wcv8 = load_w(watmp, "wcv", dt=FP8)
            for c4 in range(4):
                cs = slice(c4 * RB, (c4 + 1) * RB)
                xec = sbtmp.tile([P, KT, RB], FP8, tag="xec")
                dma_in_3d(xec, xer[:, :, cs])
                proj_f8(ppk, wck8, cv["bck"], xec, kbuf, cs)
                for ss in range(4):
                    build_v_f8(ppk, vbuf, 4 * c4 + ss, xec, ss, wcv8)

        attention(qbuf, vbuf, kbuf, cv["bcv"], False, htb)
        res2 = respool.tile([P, KT, RB], BF16, tag="res")
        proj_residual("wcp", cv["bcp"], htb, lambda m: res1[:, m, :], res2)

    # ================================ FFN =================================
    with tc.tile_pool(name="pf_stat", bufs=2, space="PSUM") as pstat, \
         tc.tile_pool(name="pf_h", bufs=2, space="PSUM") as pph, \
         tc.tile_pool(name="pf_sb", bufs=2) as sbtmp, \
         tc.tile_pool(name="pf_w", bufs=2) as watmp, \
         tc.tile_pool(name="pf_h1", bufs=1) as h1pool:
        xn3 = xnpool.tile([P, KT, RB], BF16, tag="xn2")
        ln_apply((pstat, sbtmp), res2, xn3, cv["g3"], cv["t3"])
        h1 = h1pool.tile([P, 32, RB], BF16, tag="h1")
        wf1r = d["wf1"].rearrange("(k p) m -> p k m", p=P)
        for mg in range(4):
            wg = watmp.tile([P, KT, C], BF16, tag="w")
            dma_in_3d(wg, wf1r[:, :, mg * C:(mg + 1) * C])
            for mm in range(KT):
                m = mg * KT + mm
                pp = pph.tile([P, RB], F32, tag="pp")
                for k in range(KT):
                    nc.tensor.matmul(pp, wg[:, k, mm * P:(mm + 1) * P],
                                     xn3[:, k, :],
                                     start=(k == 0), stop=(k == KT - 1))
                if m % 2 == 0:
                    nc.scalar.activation(out=h1[:, m, :], in_=pp, func=AF.Relu,
                                         bias=cv["bf1"][:, m:m + 1], scale=1.0)
                else:
                    nc.vector.tensor_scalar(out=h1[:, m, :], in0=pp,
                                            scalar1=cv["bf1"][:, m:m + 1],
                                            scalar2=0.0,
                                            op0=ALU.add, op1=ALU.max)
        wf2r = d["wf2"].rearrange("(k p) m -> p k m", p=P)
        outr = d["outT"].rearrange("(k p) q -> p k q", p=P)
        for m in range(KT):
            wg2 = watmp.tile([P, 32, P], BF16, tag="w")
            src2 = wf2r[:, :, m * P:(m + 1) * P]
            for k4 in range(4):
                nc.sync.dma_start(out=wg2[:, 8 * k4:8 * (k4 + 1), :],
                                  in_=src2[:, 8 * k4:8 * (k4 + 1), :])
            pp = pph.tile([P, RB], F32, tag="pp")
            for k in range(32):
                nc.tensor.matmul(pp, wg2[:, k, :], h1[:, k, :],
                                 start=(k == 0), stop=(k == 31))
            om = sbtmp.tile([P, RB], F32, tag="om")
            drain_sb(om, pp, 1.0, cv["bf2"][:, m:m + 1], m)
            nc.vector.tensor_add(om, om, res2[:, m, :])
            nc.sync.dma_start(out=outr[:, m, :], in_=om)

    ctx.close()


_NC_CACHE = None


def _get_nc():
    global _NC_CACHE
    if _NC_CACHE is None:
        _NC_CACHE = _build_nc()
    return _NC_CACHE


def _heads_concat(w):
    return np.ascontiguousarray(np.transpose(np.asarray(w), (1, 0, 2))
                                .reshape(C, C))


def kernel(**inputs):
    inp = {k: np.asarray(v) for k, v in inputs.items()}
    nc = _get_nc()

    def f8w(a):
        return (np.asarray(a) * WS).astype(F8)

    shared = {
        "wsq": f8w(_heads_concat(inp["mq_w"])),
        "wsk": f8w(_heads_concat(inp["mk_w"])),
        "wsv": f8w(_heads_concat(inp["mv_w"])),
        "wcq": f8w(_heads_concat(inp["cq_w"])),
        "wck": f8w(_heads_concat(inp["ck_w"])),
        "wcv": f8w(_heads_concat(inp["cv_w"])),
        "wmp": inp["m_proj_w"].astype(BF),
        "wcp": inp["c_proj_w"].astype(BF),
        "wf1": inp["f_w1"].astype(BF),
        "wf2": inp["f_w2"].astype(BF),
        "bsq": inp["mq_b"].reshape(C).astype(np.float32),
        "bsk": inp["mk_b"].reshape(C).astype(np.float32),
        "bsv": inp["mv_b"].reshape(C).astype(np.float32),
        "bcq": inp["cq_b"].reshape(C).astype(np.float32),
        "bck": inp["ck_b"].reshape(C).astype(np.float32),
        "bcv": inp["cv_b"].reshape(C).astype(np.float32),
        "bmp": inp["m_proj_b"].astype(np.float32),
        "bcp": inp["c_proj_b"].astype(np.float32),
        "bf1": inp["f_b1"].astype(np.float32),
        "bf2": inp["f_b2"].astype(np.float32),
        "g1": inp["ln1_g"].astype(np.float32),
        "t1": inp["ln1_b"].astype(np.float32),
        "g2": inp["ln2_g"].astype(np.float32),
        "t2": inp["ln2_b"].astype(np.float32),
        "g3": inp["ln3_g"].astype(np.float32),
        "t3": inp["ln3_b"].astype(np.float32),
        "smask": np.triu(np.ones((RB, RB), np.float32)).astype(BF),
    }

    x = inp["x"].astype(np.float32)
    xe = inp["x_e"].astype(np.float32)
    in_maps = []
    for core in range(NCORE):
        b, j = core // 4, core % 4
        q0 = j * RB
        perm = np.concatenate([np.arange(0, q0),
                               np.arange(q0 + RB, T),
                               np.arange(q0, q0 + RB)])
        sb = np.zeros(T, np.float32)
        sb[q0:T - RB] = NEG
        m = dict(shared)
        m["xT"] = np.ascontiguousarray(x[b][perm].T).astype(BF)
        m["xeT"] = np.ascontiguousarray(xe[b].T).astype(F8)
        m["sbias"] = sb
        in_maps.append(m)

    res = run_bass_kernel_spmd(nc, in_maps, core_ids=list(range(NCORE)))
    out = np.empty((B, T, C), np.float32)
    for core in range(NCORE):
        b, j = core // 4, core % 4
        out[b, j * RB:(j + 1) * RB, :] = res.results[core]["outT"].T
    return out
